# revision 34
# baseline (speedup 1.0000x reference)
"""Trainium2 Bass kernel for nn_COM_HGNN_K4 (heterogeneous GNN message passing).

v2 strategy (8 NeuronCores, SPMD single NEFF):
- Shard by destination nodes: each core owns 1/8 contiguous slabs
  (base 4096, joint 12288, foot 4096). x^T feature slab stays SBUF-resident.
- Encoder: data-parallel dense matmuls from host-transposed bf16 inputs.
- Message passing per layer, scatter-FIRST formulation:
    * non-transpose dma_gather pulls msg rows (edges-on-partitions) from
      replicated row-major feature tables in HBM: msg[e%128, e//128, h].
    * per 512-dst quarter: one-hot R chunks [128e, 512] built in batches on
      DVE (is_equal of fp16 winrel against a tiled iota); agg_t accumulates
      sum_e msg[e,h] R[e,d] in a PSUM bank per edge type (start=first chunk).
    * agg_t evicted to SBUF bf16 (scalar engine; mean types get a per-column
      1/deg multiply on DVE), then news += Wrel_t.T @ agg_t (dense matmul).
    * root term Wroot (premerged per dst type) initializes the news bank.
- Between layers: TensorE-transpose the slab back to row-major and
  AllGather (collective) into replicated HBM tables. Compute order
  [base, joint, foot] so AGs fire early for the next layer's first waves.
- Layer 2 skips foot-dst work, layer 3 computes base-dst only.
- Tiny decoder + symmetry coefficients applied on host.
"""

import os
import numpy as np

import concourse.bass as bass
import concourse.bacc as bacc
import concourse.tile as tile
import concourse.mybir as mybir
from concourse.masks import make_identity
from concourse.bass_utils import run_bass_kernel_spmd

# ---------------------------------------------------------------- constants
H = 128
L = 4
NB, NJ, NF = 32768, 98304, 32768
NCORES = 8
SB_B, SB_J, SB_F = NB // NCORES, NJ // NCORES, NF // NCORES  # 4096,12288,4096
SLAB = SB_B + SB_J + SB_F  # 20480
BLK = 2048          # dst nodes per block
QW = 512            # quarter width (one PSUM bank)
P = 128
THIRD = 32768       # int16-addressable joint table slice
PIECE = 4096        # rows per AG piece (x8 ranks = 32768-row piece tables)
IOTA_B = 8          # chunks per batched R build
MAX_GATHER = 8192   # max idxs per dma_gather
NQUEUES = 4         # SWDGE queues (round-robin across gathers; ucode max 4)

bf16 = mybir.dt.bfloat16
fp16 = mybir.dt.float16
f32 = mybir.dt.float32
i16 = mybir.dt.int16
np_bf16 = mybir.dt.np(bf16)

# edge types: (name, src_type, dst_type, mean?)
ETYPES = [
    ("ei_bb_gt", "base", "base", True),
    ("ei_bb_gs", "base", "base", True),
    ("ei_bb_gr", "base", "base", False),
    ("ei_bj", "base", "joint", False),
    ("ei_jb", "joint", "base", False),
    ("ei_jj", "joint", "joint", False),
    ("ei_jf", "joint", "foot", False),
    ("ei_fj", "foot", "joint", False),
]
NTYPE = {"base": NB, "joint": NJ, "foot": NF}
SLABSZ = {"base": SB_B, "joint": SB_J, "foot": SB_F}
SEGOFF = {"base": 0, "joint": SB_B, "foot": SB_B + SB_J}
MEAN_T = {0: 0, 1: 1}  # edge-type index -> dinv row (gt, gs)


def tables_for(src_type):
    if src_type == "base":
        return [0]
    if src_type == "foot":
        return [4]
    return [1, 2, 3]


# dst types processed per layer: joint first (60% of work) so its AllGather
# latency hides behind the rest; base last so AG base lands during the next
# layer's joint compute.
DSTS_PER_LAYER = [
    ["base", "joint", "foot"],
    ["base", "joint", "foot"],
    ["base", "joint"],
    ["base"],
]
# AG rounds: round r ships x^(r)
AG_TYPES = [
    ["base", "joint", "foot"],
    ["base", "joint", "foot"],
    ["base", "joint", "foot"],
    ["base", "joint"],
]
# waves per layer: (dst_type, block list)
def waves_for_layer(l):
    w = [("base", [0, 1])]
    if l < 3:
        w += [("joint", [0, 1, 2]), ("joint", [3, 4, 5])]
    if l < 2:
        w += [("foot", [0, 1])]
    return w


def _pad_feat(F):
    return ((F + 127) // 128) * 128


FB, FJ, FF = 1800, 300, 900
FBp, FJp, FFp = _pad_feat(FB), _pad_feat(FJ), _pad_feat(FF)  # 1920, 384, 1024


# ---------------------------------------------------------------- host prep
def _pack_idx_stream(idx):
    """Pack [n] indices (n % 16 == 0) into [128, n//16] int16 wrap layout."""
    n = len(idx)
    cols = n // 16
    arr = idx.reshape(cols, 16).T.astype(np.int16)
    return np.tile(arr, (8, 1))


def _build_schedule(inputs):
    """Build the per-core wave/gather/chunk schedule.

    Wave structure is identical across layers (edge sets are fixed); layer
    only changes which table round the gathers read. Returns:
      sched: per dst type: list over blocks of
             [per quarter: list of (type_slot, [chunk indices])]
      waves: list of wave dicts (dst, blocks, gathers, msg slots, chunk map)
      core arrays: idxs [128, idxcols] i16, winrel [128, nch] fp16,
                   dinv_bcast [128, 2, SB_B] bf16
    """
    # per-core, per (ti, tab): (src_idx_sorted, dst_rel_sorted, dinv_sorted)
    per_core = [dict() for _ in range(NCORES)]
    dinv_rows = [np.ones((2, SB_B), np.float32) for _ in range(NCORES)]
    for ti, (name, st, dt_, mean) in enumerate(ETYPES):
        ei = np.asarray(inputs[name])
        src, dst = ei[0].astype(np.int64), ei[1].astype(np.int64)
        if mean:
            deg = np.bincount(dst, minlength=NTYPE[dt_]).astype(np.float32)
            dinv_full = 1.0 / np.maximum(deg, 1.0)
        slab = SLABSZ[dt_]
        for c in range(NCORES):
            lo, hi = c * slab, (c + 1) * slab
            m = (dst >= lo) & (dst < hi)
            s_, d = src[m], dst[m] - lo
            if mean:
                dinv_rows[c][MEAN_T[ti]] = dinv_full[lo:hi]
            if st == "joint":
                for t3 in range(3):
                    mm = (s_ >= t3 * THIRD) & (s_ < (t3 + 1) * THIRD)
                    o = np.argsort(d[mm], kind="stable")
                    per_core[c][(ti, 1 + t3)] = ((s_[mm] - t3 * THIRD)[o], d[mm][o])
            else:
                tab = 0 if st == "base" else 4
                o = np.argsort(d, kind="stable")
                per_core[c][(ti, tab)] = (s_[o], d[o])

    # types (agg slots) per dst: list of (slot, ti, [tabs])
    DSTTYPES = {
        "base": [(0, 0, [0]), (1, 1, [0]), (2, 2, [0]), (3, 4, [1, 2, 3])],
        "joint": [(0, 3, [0]), (1, 5, [1, 2, 3]), (2, 7, [4])],
        "foot": [(0, 6, [1, 2, 3])],
    }

    idx_parts = [[] for _ in range(NCORES)]   # per-core idx cols
    winrel_parts = [[] for _ in range(NCORES)]  # per-core [P, nch] blocks, compute order
    n_chunks = 0        # global chunk counter -> winrel column (compute order)

    def build_wave(dt_, blocks):
        nonlocal n_chunks
        wave = {"dst": dt_, "blocks": blocks, "gathers": [], "nch": 0,
                "comp": []}  # comp: per (block, q): (b, q, wc0, nq, qwork)
        # gather order: joint-table tabs first (their AG lands earliest under
        # the joint-first layer order), base table last
        tab_order = {"joint": [0, 1, 2, 3, 4], "foot": [1, 2, 3],
                     "base": [0, 1, 2, 3]}[dt_]
        have = {t for (_, _, tl) in DSTTYPES[dt_] for t in tl}
        tabs = [t for t in tab_order if t in have]
        # pass 1: gather streams (tab, block, q, ti order); msg slots assigned
        slot_map = {}   # (tab, b, q, ti) -> (slot0, nch_q)
        wr_data = {}    # (tab, b, q, ti) -> per-core [P, nch_q] fp16
        msg_slot = 0
        for tab in tabs:
            g_idx = [[] for _ in range(NCORES)]
            g_len = 0       # idx slots in current gather
            g_slot0 = msg_slot

            def flush():
                nonlocal g_len, g_slot0
                if g_len == 0:
                    return
                col0 = sum(x.shape[1] for x in idx_parts[0])
                for c in range(NCORES):
                    idx_parts[c].append(
                        _pack_idx_stream(np.concatenate(g_idx[c])))
                    g_idx[c].clear()
                wave["gathers"].append((tab, col0, g_len, g_slot0))
                g_slot0 = msg_slot
                g_len = 0

            for b in blocks:
                for q in range(4):
                    lo, hi = b * BLK + q * QW, b * BLK + (q + 1) * QW
                    for (slot, ti, tl) in DSTTYPES[dt_]:
                        if tab not in tl:
                            continue
                        nch_q = 0
                        for c in range(NCORES):
                            s_, d = per_core[c][(ti, tab)]
                            m = (d >= lo) & (d < hi)
                            nch_q = max(nch_q, (int(m.sum()) + P - 1) // P)
                        if nch_q == 0:
                            continue
                        padlen = nch_q * P
                        if g_len + padlen > MAX_GATHER:
                            flush()
                        wrs = []
                        for c in range(NCORES):
                            s_, d = per_core[c][(ti, tab)]
                            m = (d >= lo) & (d < hi)
                            sp = np.zeros(padlen, np.int64)
                            wp = np.full(padlen, -1000.0, np.float32)
                            k = int(m.sum())
                            sp[:k] = s_[m]
                            wp[:k] = (d[m] - b * BLK - q * QW).astype(np.float32)
                            g_idx[c].append(sp)
                            wrs.append(wp.reshape(nch_q, P).T.astype(np.float16))
                        slot_map[(tab, b, q, ti)] = (msg_slot, nch_q)
                        wr_data[(tab, b, q, ti)] = wrs
                        msg_slot += nch_q
                        g_len += padlen
            flush()
        wave["nch"] = msg_slot
        # pass 2: compute plan; winrel columns assigned in compute order so
        # each quarter's chunks occupy a contiguous winrel range
        for b in blocks:
            for q in range(4):
                wc0 = n_chunks
                qwork = []
                for (slot, ti, tl) in DSTTYPES[dt_]:
                    chunks = []
                    for tab in tl:
                        e = slot_map.get((tab, b, q, ti))
                        if e is None:
                            continue
                        s0, nch_q = e
                        for c in range(NCORES):
                            winrel_parts[c].append(wr_data[(tab, b, q, ti)][c])
                        chunks += [(s0 + k, n_chunks + k) for k in range(nch_q)]
                        n_chunks += nch_q
                    if chunks:
                        qwork.append((slot, ti, chunks))
                wave["comp"].append((b, q, wc0, n_chunks - wc0, qwork))
        return wave

    # one wave set (edge structure identical across layers)
    wave_sets = {}
    for dt_, blocks in [("base", [0]), ("base", [1]), ("joint", [0, 1, 2]),
                        ("joint", [3, 4, 5]), ("foot", [0, 1])]:
        wave_sets.setdefault(dt_, []).append(build_wave(dt_, blocks))

    core_arrays = []
    idxcols = sum(x.shape[1] for x in idx_parts[0])
    for c in range(NCORES):
        core_arrays.append({
            "idxs": np.concatenate(idx_parts[c], axis=1),
            "winrel": np.concatenate(winrel_parts[c], axis=1),
            "dinv": np.repeat(
                np.stack(dinv_rows[c]).reshape(1, 2 * SB_B), P, axis=0
            ).astype(np_bf16),
        })
        assert core_arrays[c]["idxs"].shape[1] == idxcols
        assert core_arrays[c]["winrel"].shape[1] == n_chunks
    return wave_sets, core_arrays, n_chunks, idxcols


# ---------------------------------------------------------------- device build
def _build(nc, wave_sets, n_chunks, idxcols, max_wave_nch, max_q_nch):
    dram_in = {}

    def din(name, shape, dtype):
        dram_in[name] = nc.dram_tensor(name, shape, dtype, kind="ExternalInput").ap()
        return dram_in[name]

    xraw = {
        "base": din("xT_base", [FBp, SB_B], bf16),
        "joint": din("xT_joint", [FJp, SB_J], bf16),
        "foot": din("xT_foot", [FFp, SB_F], bf16),
    }
    encw = {
        "base": din("encWT_base", [P, FBp], bf16),
        "joint": din("encWT_joint", [P, FJp], bf16),
        "foot": din("encWT_foot", [P, FFp], bf16),
    }
    encb = din("enc_b", [P, 3], f32)  # columns: base, joint, foot
    wrelT = din("wrelT", [P, L * 8 * P], bf16)       # [h, l*8*128]
    wrootT = din("wrootT", [P, L * 3 * P], bf16)     # [h, l*3*128] (b,j,f)
    brel = din("brelsum", [P, L * 3], f32)
    btw1T = din("btW1T", [P, P], bf16)
    btw2T = din("btW2T", [P, P], bf16)
    btb = din("btb", [P, 2], f32)
    idxs_d = din("idxs", [P, idxcols], i16)
    winrel_d = din("winrel", [P, n_chunks], fp16)
    dinv_d = din("dinv", [P, 2 * SB_B], bf16)
    iota_d = din("iotat", [P, IOTA_B, QW], fp16)

    out_xb = nc.dram_tensor("out_xbase", [SB_B, P], f32, kind="ExternalOutput").ap()
    gather_q = [0]

    with tile.TileContext(nc) as tc:
        with (
            tc.tile_pool(name="const", bufs=1) as cp,
            tc.tile_pool(name="sb", bufs=2) as sb,
            tc.tile_pool(name="ps", bufs=1, space="PSUM") as psn,
            tc.tile_pool(name="psy", bufs=2, space="PSUM") as psy,
            tc.tile_pool(name="dram", bufs=1, space="DRAM") as dram,
        ):
            # ---------------- constants into SBUF
            xT = cp.tile([P, SLAB], bf16)
            idxs_sb = cp.tile([P, idxcols], i16)
            nc.sync.dma_start(idxs_sb[:], idxs_d[:])
            winrel_sb = cp.tile([P, n_chunks], fp16)
            nc.sync.dma_start(winrel_sb[:], winrel_d[:])
            iota_sb = cp.tile([P, IOTA_B, QW], fp16)
            nc.sync.dma_start(iota_sb[:], iota_d[:])
            wrel_sb = cp.tile([P, L * 8 * P], bf16)
            nc.sync.dma_start(wrel_sb[:], wrelT[:])
            wroot_sb = cp.tile([P, L * 3 * P], bf16)
            nc.sync.dma_start(wroot_sb[:], wrootT[:])
            brel_sb = cp.tile([P, L * 3], f32)
            nc.sync.dma_start(brel_sb[:], brel[:])
            btw1_sb = cp.tile([P, P], bf16)
            nc.sync.dma_start(btw1_sb[:], btw1T[:])
            btw2_sb = cp.tile([P, P], bf16)
            nc.sync.dma_start(btw2_sb[:], btw2T[:])
            btb_sb = cp.tile([P, 2], f32)
            nc.sync.dma_start(btb_sb[:], btb[:])
            encb_sb = cp.tile([P, 3], f32)
            nc.sync.dma_start(encb_sb[:], encb[:])
            encw_sb = {}
            for dt_ in ("base", "joint", "foot"):
                t = cp.tile([P, encw[dt_].shape[1]], bf16, name=f"encw_{dt_}")
                nc.sync.dma_start(t[:], encw[dt_][:])
                encw_sb[dt_] = t
            ident = cp.tile([P, P], bf16)
            make_identity(nc, ident[:])

            # ---------------- DRAM internals
            agout = []
            for rnd in range(L):
                agout.append(
                    {
                        dt_: dram.tile(
                            [NTYPE[dt_], P], bf16, name=f"ago_{dt_}{rnd}",
                            addr_space="Shared",
                        )
                        for dt_ in AG_TYPES[rnd]
                    }
                )
            agin = {
                "base": dram.tile([SB_B, P], bf16, name="agi_b"),
                "joint": dram.tile([SB_J, P], bf16, name="agi_j"),
                "foot": dram.tile([SB_F, P], bf16, name="agi_f"),
            }

            # ---------------- helper: transpose 2048 rows into agin; on the
            # piece boundary, AllGather the 4096-row piece into its rank-major
            # piece table (pipelines comm through the layer)
            def ship_block(round_idx, dt_, lo, hi):
                seg = SEGOFF[dt_]
                for wt in range(lo // P, hi // P):
                    trp = psy.tile([P, P], bf16, tag="y", name=f"trp{round_idx}{dt_}{wt}")
                    nc.tensor.transpose(
                        out=trp[:],
                        in_=xT[:, seg + wt * P : seg + (wt + 1) * P],
                        identity=ident[:],
                    )
                    trs = sb.tile([P, P], bf16, tag="trs", bufs=3)
                    nc.scalar.copy(out=trs[:], in_=trp[:])
                    nc.sync.dma_start(
                        agin[dt_][wt * P : (wt + 1) * P, :], trs[:]
                    )
                if hi != SLABSZ[dt_] or os.environ.get("HGNN_NO_CC"):
                    return
                nc.gpsimd.collective_compute(
                    "AllGather",
                    mybir.AluOpType.bypass,
                    replica_groups=[list(range(NCORES))],
                    ins=[agin[dt_][:].opt()],
                    outs=[agout[round_idx][dt_][:].opt()],
                )

            # ---------------- encoder (order: joint, foot, base for AG timing)
            FP = {"base": FBp, "joint": FJp, "foot": FFp}
            for dt_ in ["base", "joint", "foot"]:
                Fp, seg = FP[dt_], SEGOFF[dt_]
                ntiles = SLABSZ[dt_] // QW
                bcol = {"base": 0, "joint": 1, "foot": 2}[dt_]
                for nt in range(ntiles):
                    acc = psn.tile([P, QW], f32, tag="news", bufs=2)
                    for fc in range(Fp // P):
                        rhs = sb.tile([P, QW], bf16, tag="encrhs", bufs=3)
                        nc.sync.dma_start(
                            rhs[:], xraw[dt_][fc * P : (fc + 1) * P, nt * QW : (nt + 1) * QW]
                        )
                        nc.tensor.matmul(
                            out=acc[:],
                            lhsT=encw_sb[dt_][:, fc * P : (fc + 1) * P],
                            rhs=rhs[:],
                            start=(fc == 0),
                            stop=(fc == Fp // P - 1),
                        )
                    nc.scalar.activation(
                        out=xT[:, seg + nt * QW : seg + (nt + 1) * QW],
                        in_=acc[:],
                        func=mybir.ActivationFunctionType.Relu,
                        bias=encb_sb[:, bcol : bcol + 1],
                    )
                    if (nt + 1) * QW % BLK == 0:
                        ship_block(0, dt_, (nt + 1) * QW - BLK, (nt + 1) * QW)

            # ---------------- layers
            for l in range(L):
                for dt_ in DSTS_PER_LAYER[l]:
                    wlist = wave_sets[dt_]
                    seg = SEGOFF[dt_]
                    dcol = {"base": 0, "joint": 1, "foot": 2}[dt_]
                    wroot = wroot_sb[:, (l * 3 + dcol) * P : (l * 3 + dcol + 1) * P]
                    bias = brel_sb[:, l * 3 + dcol : l * 3 + dcol + 1]
                    for wave in wlist:
                        # gathers for this wave
                        msg = sb.tile(
                            [P, max_wave_nch, P], bf16, tag="msg", bufs=2,
                            name=f"m{l}{dt_}{wave['blocks'][0]}",
                        )
                        for gi, (tab, icol, n, slot0) in enumerate(wave["gathers"]):
                            if tab == 0:
                                src_ap = agout[l]["base"][:]
                            elif tab == 4:
                                src_ap = agout[l]["foot"][:]
                            else:
                                t3 = tab - 1
                                src_ap = agout[l]["joint"][
                                    t3 * THIRD : (t3 + 1) * THIRD, :
                                ]
                            if os.environ.get("HGNN_NO_GATHER"):
                                continue
                            nc.gpsimd.dma_gather(
                                out_ap=msg[:, slot0 : slot0 + n // P, :],
                                in_ap=src_ap,
                                idxs_ap=idxs_sb[:, icol : icol + n // 16],
                                num_idxs=n,
                                num_idxs_reg=n,
                                elem_size=P,
                                transpose=False,
                                single_packet=False,
                                queue_num=gather_q[0] % NQUEUES,
                            )
                            gather_q[0] += 1
                        # compute per (block, quarter)
                        for (b, q, wc0, nq, qwork) in wave["comp"]:
                            base_col = seg + b * BLK + q * QW
                            news = psn.tile(
                                [P, QW], f32, tag="news", bufs=2,
                                name=f"n{l}{dt_}{b}{q}",
                            )
                            nc.tensor.matmul(
                                out=news[:],
                                lhsT=wroot,
                                rhs=xT[:, base_col : base_col + QW],
                                start=True,
                                stop=False,
                            )
                            if qwork:
                                # batched R builds over contiguous winrel cols
                                Rq = sb.tile(
                                    [P, max_q_nch, QW], bf16, tag="R", bufs=2,
                                    name=f"R{l}{dt_}{b}{q}",
                                )
                                for i0 in range(0, nq, IOTA_B):
                                    i1 = min(i0 + IOTA_B, nq)
                                    nc.vector.tensor_tensor(
                                        out=Rq[:, i0:i1, :],
                                        in0=winrel_sb[:, wc0 + i0 : wc0 + i1]
                                        .unsqueeze(2)
                                        .to_broadcast([P, i1 - i0, QW]),
                                        in1=iota_sb[:, : i1 - i0, :],
                                        op=mybir.AluOpType.is_equal,
                                    )
                            for (slot, ti, chunks) in qwork:
                                agg = psn.tile(
                                    [P, QW], f32, tag=f"agg{slot}", bufs=1,
                                    name=f"a{l}{dt_}{b}{q}{slot}",
                                )
                                for k, (mslot, wcol) in enumerate(chunks):
                                    nc.tensor.matmul(
                                        out=agg[:],
                                        lhsT=msg[:, mslot, :],
                                        rhs=Rq[:, wcol - wc0, :],
                                        start=(k == 0),
                                        stop=False,
                                    )
                                aggsb = sb.tile(
                                    [P, QW], bf16, tag="aggsb", bufs=4
                                )
                                if ti in MEAN_T:
                                    dv = sb.tile(
                                        [P, QW], bf16, tag="dinv", bufs=2
                                    )
                                    dc0 = MEAN_T[ti] * SB_B + b * BLK + q * QW
                                    nc.sync.dma_start(
                                        dv[:], dinv_d[:, dc0 : dc0 + QW]
                                    )
                                    nc.vector.tensor_tensor(
                                        out=aggsb[:],
                                        in0=agg[:],
                                        in1=dv[:],
                                        op=mybir.AluOpType.mult,
                                    )
                                else:
                                    nc.scalar.copy(out=aggsb[:], in_=agg[:])
                                nc.tensor.matmul(
                                    out=news[:],
                                    lhsT=wrel_sb[:, (l * 8 + ti) * P : (l * 8 + ti + 1) * P],
                                    rhs=aggsb[:],
                                    start=False,
                                    stop=False,
                                )
                            # eviction
                            cols = slice(base_col, base_col + QW)
                            if dt_ != "base":
                                tmp = sb.tile([P, QW], bf16, tag="ev", bufs=3)
                                nc.scalar.activation(
                                    out=tmp[:],
                                    in_=news[:],
                                    func=mybir.ActivationFunctionType.Relu,
                                    bias=bias,
                                )
                                nc.vector.tensor_tensor(
                                    out=xT[:, cols],
                                    in0=xT[:, cols],
                                    in1=tmp[:],
                                    op=mybir.AluOpType.add,
                                )
                            else:
                                nb_ = sb.tile([P, QW], bf16, tag="ev", bufs=3)
                                nc.scalar.activation(
                                    out=nb_[:],
                                    in_=news[:],
                                    func=mybir.ActivationFunctionType.Identity,
                                    bias=bias,
                                )
                                t1p = psy.tile([P, QW], f32, tag="y", name=f"t1{l}{b}{q}")
                                nc.tensor.matmul(
                                    out=t1p[:], lhsT=btw1_sb[:], rhs=nb_[:],
                                    start=True, stop=True,
                                )
                                t1s = sb.tile([P, QW], bf16, tag="ev2", bufs=3)
                                nc.scalar.activation(
                                    out=t1s[:], in_=t1p[:],
                                    func=mybir.ActivationFunctionType.Relu,
                                    bias=btb_sb[:, 0:1],
                                )
                                t2p = psy.tile([P, QW], f32, tag="y", name=f"t2{l}{b}{q}")
                                nc.tensor.matmul(
                                    out=t2p[:], lhsT=btw2_sb[:], rhs=t1s[:],
                                    start=True, stop=True,
                                )
                                t2s = sb.tile([P, QW], bf16, tag="ev2", bufs=3)
                                nc.scalar.activation(
                                    out=t2s[:], in_=t2p[:],
                                    func=mybir.ActivationFunctionType.Identity,
                                    bias=btb_sb[:, 1:2],
                                )
                                if l < L - 1:
                                    nc.vector.tensor_tensor(
                                        out=xT[:, cols],
                                        in0=xT[:, cols],
                                        in1=t2s[:],
                                        op=mybir.AluOpType.add,
                                    )
                                else:
                                    fin = sb.tile([P, QW], bf16, tag="ev", bufs=3)
                                    nc.vector.tensor_tensor(
                                        out=fin[:],
                                        in0=xT[:, cols],
                                        in1=t2s[:],
                                        op=mybir.AluOpType.add,
                                    )
                                    for wt in range(QW // P):
                                        ftp = psy.tile([P, P], bf16, tag="y", name=f"f{b}{q}{wt}")
                                        nc.tensor.transpose(
                                            out=ftp[:],
                                            in_=fin[:, wt * P : (wt + 1) * P],
                                            identity=ident[:],
                                        )
                                        fts = sb.tile([P, P], f32, tag="fts", bufs=3)
                                        nc.scalar.copy(out=fts[:], in_=ftp[:])
                                        r0 = b * BLK + q * QW + wt * P
                                        nc.sync.dma_start(
                                            out_xb[r0 : r0 + P, :], fts[:]
                                        )
                            # ship the finished 2048-row block as its own
                            # AllGather piece (pipelines comm through the layer)
                            if q == 3 and l < L - 1 and dt_ in AG_TYPES[l + 1]:
                                ship_block(l + 1, dt_, b * BLK, (b + 1) * BLK)

    return dram_in


# ---------------------------------------------------------------- main entry
def kernel(**inputs):
    xb = np.asarray(inputs["x_base"], np.float32)
    xj = np.asarray(inputs["x_joint"], np.float32)
    xf = np.asarray(inputs["x_foot"], np.float32)

    wave_sets, core_arrays, n_chunks, idxcols = _build_schedule(inputs)
    max_wave_nch = max(w["nch"] for ws in wave_sets.values() for w in ws)
    max_q_nch = max(
        nq for ws in wave_sets.values() for w in ws
        for (b, q, wc0, nq, qwork) in w["comp"]
    )

    nc = bacc.Bacc("TRN2", target_bir_lowering=False, debug=False, num_devices=NCORES,
                   num_swdge_queues=NQUEUES)
    _build(nc, wave_sets, n_chunks, idxcols, max_wave_nch, max_q_nch)
    nc.compile()

    # ---- per-core inputs
    def padT(x, Fp):
        out = np.zeros((Fp, x.shape[0]), np_bf16)
        out[: x.shape[1]] = np.ascontiguousarray(x.T).astype(np_bf16)
        return out

    def enc_pack(W, Fp):
        WT = np.zeros((Fp, P), np.float32)
        WT[: W.shape[1]] = W.T
        return (
            WT.reshape(Fp // P, P, P).transpose(1, 0, 2).reshape(P, Fp).astype(np_bf16)
        )

    wrel = np.asarray(inputs["conv_Wrel"], np.float32)   # [L, 8, H, H]
    wroot = np.asarray(inputs["conv_Wroot"], np.float32)
    brel = np.asarray(inputs["conv_brel"], np.float32)   # [L, 8, H]
    wrelT = (
        wrel.transpose(0, 1, 3, 2).reshape(L * 8, P, P).transpose(1, 0, 2).reshape(P, L * 8 * P)
    ).astype(np_bf16)
    wrootT = np.zeros((P, L * 3 * P), np.float32)
    brelsum = np.zeros((P, L * 3), np.float32)
    for l in range(L):
        for di, dt_ in enumerate(["base", "joint", "foot"]):
            wsum = np.zeros((P, P), np.float32)
            bsum = np.zeros(P, np.float32)
            for ti, (nm, st, d2, mn) in enumerate(ETYPES):
                if d2 == dt_:
                    wsum += wroot[l, ti]
                    bsum += brel[l, ti]
            wrootT[:, (l * 3 + di) * P : (l * 3 + di + 1) * P] = wsum.T
            brelsum[:, l * 3 + di] = bsum
    wrootT = wrootT.astype(np_bf16)

    encb = np.stack(
        [
            np.asarray(inputs["enc_b_base"], np.float32),
            np.asarray(inputs["enc_b_joint"], np.float32),
            np.asarray(inputs["enc_b_foot"], np.float32),
        ],
        axis=1,
    )
    btb = np.stack(
        [
            np.asarray(inputs["bt_b1"], np.float32),
            np.asarray(inputs["bt_b2"], np.float32),
        ],
        axis=1,
    )
    iota_tiled = np.tile(
        np.arange(QW, dtype=np.float16), IOTA_B
    ).reshape(1, IOTA_B, QW).repeat(P, 0)

    common = {
        "encWT_base": enc_pack(np.asarray(inputs["enc_W_base"], np.float32), FBp),
        "encWT_joint": enc_pack(np.asarray(inputs["enc_W_joint"], np.float32), FJp),
        "encWT_foot": enc_pack(np.asarray(inputs["enc_W_foot"], np.float32), FFp),
        "enc_b": encb,
        "wrelT": wrelT,
        "wrootT": wrootT,
        "brelsum": brelsum,
        "btW1T": np.asarray(inputs["bt_W1"], np.float32).T.astype(np_bf16).copy(),
        "btW2T": np.asarray(inputs["bt_W2"], np.float32).T.astype(np_bf16).copy(),
        "btb": btb,
        "iotat": iota_tiled,
    }

    in_maps = []
    for c in range(NCORES):
        m = dict(common)
        m["xT_base"] = padT(xb[c * SB_B : (c + 1) * SB_B], FBp)
        m["xT_joint"] = padT(xj[c * SB_J : (c + 1) * SB_J], FJp)
        m["xT_foot"] = padT(xf[c * SB_F : (c + 1) * SB_F], FFp)
        m["idxs"] = core_arrays[c]["idxs"]
        m["winrel"] = core_arrays[c]["winrel"]
        m["dinv"] = core_arrays[c]["dinv"]
        in_maps.append(m)

    trace = bool(os.environ.get("HGNN_TRACE"))
    res = run_bass_kernel_spmd(
        nc, in_maps, core_ids=list(range(NCORES)), trace=trace
    )
    if res.exec_time_ns is not None:
        print(f"HW exec time: {res.exec_time_ns} ns", flush=True)
    xbase_fin = np.concatenate(
        [res.results[c]["out_xbase"] for c in range(NCORES)], axis=0
    )  # [32768, 128] fp32

    # host decoder (tiny)
    dec_W = np.asarray(inputs["dec_W"], np.float32)
    dec_b = np.asarray(inputs["dec_b"], np.float32)
    coeff_lin = np.asarray(inputs["coeff_lin"], np.float32)
    coeff_ang = np.asarray(inputs["coeff_ang"], np.float32)
    bs = NB // 4
    out = xbase_fin.reshape(bs, 4 * H) @ dec_W.T + dec_b
    xr = out.reshape(bs, 4, 6)
    x_lin = (xr[:, :, :3].reshape(bs, 12) * coeff_lin).reshape(bs, 4, 3)
    x_ang = (xr[:, :, 3:].reshape(bs, 12) * coeff_ang).reshape(bs, 4, 3)
    return np.concatenate([x_lin, x_ang], axis=-1).reshape(bs, 24).astype(np.float32)


# revision 35
# speedup vs baseline: 1.0224x; 1.0224x over previous
"""Trainium2 Bass kernel for nn_COM_HGNN_K4 (heterogeneous GNN message passing).

v2 strategy (8 NeuronCores, SPMD single NEFF):
- Shard by destination nodes: each core owns 1/8 contiguous slabs
  (base 4096, joint 12288, foot 4096). x^T feature slab stays SBUF-resident.
- Encoder: data-parallel dense matmuls from host-transposed bf16 inputs.
- Message passing per layer, scatter-FIRST formulation:
    * non-transpose dma_gather pulls msg rows (edges-on-partitions) from
      replicated row-major feature tables in HBM: msg[e%128, e//128, h].
    * per 512-dst quarter: one-hot R chunks [128e, 512] built in batches on
      DVE (is_equal of fp16 winrel against a tiled iota); agg_t accumulates
      sum_e msg[e,h] R[e,d] in a PSUM bank per edge type (start=first chunk).
    * agg_t evicted to SBUF bf16 (scalar engine; mean types get a per-column
      1/deg multiply on DVE), then news += Wrel_t.T @ agg_t (dense matmul).
    * root term Wroot (premerged per dst type) initializes the news bank.
- Between layers: TensorE-transpose the slab back to row-major and
  AllGather (collective) into replicated HBM tables. Compute order
  [base, joint, foot] so AGs fire early for the next layer's first waves.
- Layer 2 skips foot-dst work, layer 3 computes base-dst only.
- Tiny decoder + symmetry coefficients applied on host.
"""

import os
import numpy as np

import concourse.bass as bass
import concourse.bacc as bacc
import concourse.tile as tile
import concourse.mybir as mybir
from concourse.masks import make_identity
from concourse.bass_utils import run_bass_kernel_spmd

# ---------------------------------------------------------------- constants
H = 128
L = 4
NB, NJ, NF = 32768, 98304, 32768
NCORES = 8
SB_B, SB_J, SB_F = NB // NCORES, NJ // NCORES, NF // NCORES  # 4096,12288,4096
SLAB = SB_B + SB_J + SB_F  # 20480
BLK = 2048          # dst nodes per block
QW = 512            # quarter width (one PSUM bank)
P = 128
THIRD = 32768       # int16-addressable joint table slice
PIECE = 4096        # rows per AG piece (x8 ranks = 32768-row piece tables)
IOTA_B = 8          # chunks per batched R build
MAX_GATHER = 8192   # max idxs per dma_gather
NQUEUES = 4         # SWDGE queues (round-robin across gathers; ucode max 4)

bf16 = mybir.dt.bfloat16
fp16 = mybir.dt.float16
f32 = mybir.dt.float32
i16 = mybir.dt.int16
np_bf16 = mybir.dt.np(bf16)

# edge types: (name, src_type, dst_type, mean?)
ETYPES = [
    ("ei_bb_gt", "base", "base", True),
    ("ei_bb_gs", "base", "base", True),
    ("ei_bb_gr", "base", "base", False),
    ("ei_bj", "base", "joint", False),
    ("ei_jb", "joint", "base", False),
    ("ei_jj", "joint", "joint", False),
    ("ei_jf", "joint", "foot", False),
    ("ei_fj", "foot", "joint", False),
]
NTYPE = {"base": NB, "joint": NJ, "foot": NF}
SLABSZ = {"base": SB_B, "joint": SB_J, "foot": SB_F}
SEGOFF = {"base": 0, "joint": SB_B, "foot": SB_B + SB_J}
MEAN_T = {0: 0, 1: 1}  # edge-type index -> dinv row (gt, gs)


def tables_for(src_type):
    if src_type == "base":
        return [0]
    if src_type == "foot":
        return [4]
    return [1, 2, 3]


# dst types processed per layer: joint first (60% of work) so its AllGather
# latency hides behind the rest; base last so AG base lands during the next
# layer's joint compute.
DSTS_PER_LAYER = [
    ["base", "joint", "foot"],
    ["base", "joint", "foot"],
    ["base", "joint"],
    ["base"],
]
# AG rounds: round r ships x^(r)
AG_TYPES = [
    ["base", "joint", "foot"],
    ["base", "joint", "foot"],
    ["base", "joint", "foot"],
    ["base", "joint"],
]
# waves per layer: (dst_type, block list)
def waves_for_layer(l):
    w = [("base", [0, 1])]
    if l < 3:
        w += [("joint", [0, 1, 2]), ("joint", [3, 4, 5])]
    if l < 2:
        w += [("foot", [0, 1])]
    return w


def _pad_feat(F):
    return ((F + 127) // 128) * 128


FB, FJ, FF = 1800, 300, 900
FBp, FJp, FFp = _pad_feat(FB), _pad_feat(FJ), _pad_feat(FF)  # 1920, 384, 1024


# ---------------------------------------------------------------- host prep
def _pack_idx_stream(idx):
    """Pack [n] indices (n % 16 == 0) into [128, n//16] int16 wrap layout."""
    n = len(idx)
    cols = n // 16
    arr = idx.reshape(cols, 16).T.astype(np.int16)
    return np.tile(arr, (8, 1))


def _build_schedule(inputs):
    """Build the per-core wave/gather/chunk schedule.

    Wave structure is identical across layers (edge sets are fixed); layer
    only changes which table round the gathers read. Returns:
      sched: per dst type: list over blocks of
             [per quarter: list of (type_slot, [chunk indices])]
      waves: list of wave dicts (dst, blocks, gathers, msg slots, chunk map)
      core arrays: idxs [128, idxcols] i16, winrel [128, nch] fp16,
                   dinv_bcast [128, 2, SB_B] bf16
    """
    # per-core, per (ti, tab): (src_idx_sorted, dst_rel_sorted, dinv_sorted)
    per_core = [dict() for _ in range(NCORES)]
    dinv_rows = [np.ones((2, SB_B), np.float32) for _ in range(NCORES)]
    for ti, (name, st, dt_, mean) in enumerate(ETYPES):
        ei = np.asarray(inputs[name])
        src, dst = ei[0].astype(np.int64), ei[1].astype(np.int64)
        if mean:
            deg = np.bincount(dst, minlength=NTYPE[dt_]).astype(np.float32)
            dinv_full = 1.0 / np.maximum(deg, 1.0)
        slab = SLABSZ[dt_]
        for c in range(NCORES):
            lo, hi = c * slab, (c + 1) * slab
            m = (dst >= lo) & (dst < hi)
            s_, d = src[m], dst[m] - lo
            if mean:
                dinv_rows[c][MEAN_T[ti]] = dinv_full[lo:hi]
            if st == "joint":
                for t3 in range(3):
                    mm = (s_ >= t3 * THIRD) & (s_ < (t3 + 1) * THIRD)
                    o = np.argsort(d[mm], kind="stable")
                    per_core[c][(ti, 1 + t3)] = ((s_[mm] - t3 * THIRD)[o], d[mm][o])
            else:
                tab = 0 if st == "base" else 4
                o = np.argsort(d, kind="stable")
                per_core[c][(ti, tab)] = (s_[o], d[o])

    # types (agg slots) per dst: list of (slot, ti, [tabs])
    DSTTYPES = {
        "base": [(0, 0, [0]), (1, 1, [0]), (2, 2, [0]), (3, 4, [1, 2, 3])],
        "joint": [(0, 3, [0]), (1, 5, [1, 2, 3]), (2, 7, [4])],
        "foot": [(0, 6, [1, 2, 3])],
    }

    idx_parts = [[] for _ in range(NCORES)]   # per-core idx cols
    winrel_parts = [[] for _ in range(NCORES)]  # per-core [P, nch] blocks, compute order
    n_chunks = 0        # global chunk counter -> winrel column (compute order)

    def build_wave(dt_, blocks):
        nonlocal n_chunks
        wave = {"dst": dt_, "blocks": blocks, "gathers": [], "nch": 0,
                "comp": []}  # comp: per (block, q): (b, q, wc0, nq, qwork)
        # gather order: joint-table tabs first (their AG lands earliest under
        # the joint-first layer order), base table last
        tab_order = {"joint": [0, 1, 2, 3, 4], "foot": [1, 2, 3],
                     "base": [0, 1, 2, 3]}[dt_]
        have = {t for (_, _, tl) in DSTTYPES[dt_] for t in tl}
        tabs = [t for t in tab_order if t in have]
        # pass 1: gather streams (tab, block, q, ti order); msg slots assigned
        slot_map = {}   # (tab, b, q, ti) -> (slot0, nch_q)
        wr_data = {}    # (tab, b, q, ti) -> per-core [P, nch_q] fp16
        msg_slot = 0
        for tab in tabs:
            g_idx = [[] for _ in range(NCORES)]
            g_len = 0       # idx slots in current gather
            g_slot0 = msg_slot

            def flush():
                nonlocal g_len, g_slot0
                if g_len == 0:
                    return
                col0 = sum(x.shape[1] for x in idx_parts[0])
                for c in range(NCORES):
                    idx_parts[c].append(
                        _pack_idx_stream(np.concatenate(g_idx[c])))
                    g_idx[c].clear()
                wave["gathers"].append((tab, col0, g_len, g_slot0))
                g_slot0 = msg_slot
                g_len = 0

            for b in blocks:
                for q in range(4):
                    lo, hi = b * BLK + q * QW, b * BLK + (q + 1) * QW
                    for (slot, ti, tl) in DSTTYPES[dt_]:
                        if tab not in tl:
                            continue
                        nch_q = 0
                        for c in range(NCORES):
                            s_, d = per_core[c][(ti, tab)]
                            m = (d >= lo) & (d < hi)
                            nch_q = max(nch_q, (int(m.sum()) + P - 1) // P)
                        if nch_q == 0:
                            continue
                        padlen = nch_q * P
                        if g_len + padlen > MAX_GATHER:
                            flush()
                        wrs = []
                        for c in range(NCORES):
                            s_, d = per_core[c][(ti, tab)]
                            m = (d >= lo) & (d < hi)
                            sp = np.zeros(padlen, np.int64)
                            wp = np.full(padlen, -1000.0, np.float32)
                            k = int(m.sum())
                            sp[:k] = s_[m]
                            wp[:k] = (d[m] - b * BLK - q * QW).astype(np.float32)
                            g_idx[c].append(sp)
                            wrs.append(wp.reshape(nch_q, P).T.astype(np.float16))
                        slot_map[(tab, b, q, ti)] = (msg_slot, nch_q)
                        wr_data[(tab, b, q, ti)] = wrs
                        msg_slot += nch_q
                        g_len += padlen
            flush()
        wave["nch"] = msg_slot
        # pass 2: compute plan; winrel columns assigned in compute order so
        # each quarter's chunks occupy a contiguous winrel range
        for b in blocks:
            for q in range(4):
                wc0 = n_chunks
                qwork = []
                for (slot, ti, tl) in DSTTYPES[dt_]:
                    chunks = []
                    for tab in tl:
                        e = slot_map.get((tab, b, q, ti))
                        if e is None:
                            continue
                        s0, nch_q = e
                        for c in range(NCORES):
                            winrel_parts[c].append(wr_data[(tab, b, q, ti)][c])
                        chunks += [(s0 + k, n_chunks + k) for k in range(nch_q)]
                        n_chunks += nch_q
                    if chunks:
                        qwork.append((slot, ti, chunks))
                wave["comp"].append((b, q, wc0, n_chunks - wc0, qwork))
        return wave

    # one wave set (edge structure identical across layers)
    wave_sets = {}
    for dt_, blocks in [("base", [0]), ("base", [1]), ("joint", [0, 1, 2]),
                        ("joint", [3, 4, 5]), ("foot", [0, 1])]:
        wave_sets.setdefault(dt_, []).append(build_wave(dt_, blocks))

    core_arrays = []
    idxcols = sum(x.shape[1] for x in idx_parts[0])
    for c in range(NCORES):
        core_arrays.append({
            "idxs": np.concatenate(idx_parts[c], axis=1),
            "winrel": np.concatenate(winrel_parts[c], axis=1),
            "dinv": np.repeat(
                np.stack(dinv_rows[c]).reshape(1, 2 * SB_B), P, axis=0
            ).astype(np_bf16),
        })
        assert core_arrays[c]["idxs"].shape[1] == idxcols
        assert core_arrays[c]["winrel"].shape[1] == n_chunks
    return wave_sets, core_arrays, n_chunks, idxcols


# ---------------------------------------------------------------- device build
def _build(nc, wave_sets, n_chunks, idxcols, max_wave_nch, max_q_nch):
    dram_in = {}

    def din(name, shape, dtype):
        dram_in[name] = nc.dram_tensor(name, shape, dtype, kind="ExternalInput").ap()
        return dram_in[name]

    xraw = {
        "base": din("xT_base", [FBp, SB_B], bf16),
        "joint": din("xT_joint", [FJp, SB_J], bf16),
        "foot": din("xT_foot", [FFp, SB_F], bf16),
    }
    encw = {
        "base": din("encWT_base", [P, FBp], bf16),
        "joint": din("encWT_joint", [P, FJp], bf16),
        "foot": din("encWT_foot", [P, FFp], bf16),
    }
    encb = din("enc_b", [P, 3], f32)  # columns: base, joint, foot
    wrelT = din("wrelT", [P, L * 8 * P], bf16)       # [h, l*8*128]
    wrootT = din("wrootT", [P, L * 3 * P], bf16)     # [h, l*3*128] (b,j,f)
    brel = din("brelsum", [P, L * 3], f32)
    btw1T = din("btW1T", [P, P], bf16)
    btw2T = din("btW2T", [P, P], bf16)
    btb = din("btb", [P, 2], f32)
    idxs_d = din("idxs", [P, idxcols], i16)
    winrel_d = din("winrel", [P, n_chunks], fp16)
    dinv_d = din("dinv", [P, 2 * SB_B], bf16)
    iota_d = din("iotat", [P, IOTA_B, QW], fp16)

    out_xb = nc.dram_tensor("out_xbase", [SB_B, P], f32, kind="ExternalOutput").ap()
    gather_q = [0]

    with tile.TileContext(nc) as tc:
        with (
            tc.tile_pool(name="const", bufs=1) as cp,
            tc.tile_pool(name="sb", bufs=2) as sb,
            tc.tile_pool(name="ps", bufs=1, space="PSUM") as psn,
            tc.tile_pool(name="psy", bufs=2, space="PSUM") as psy,
            tc.tile_pool(name="dram", bufs=1, space="DRAM") as dram,
        ):
            # ---------------- constants into SBUF
            xT = cp.tile([P, SLAB], bf16)
            idxs_sb = cp.tile([P, idxcols], i16)
            nc.sync.dma_start(idxs_sb[:], idxs_d[:])
            winrel_sb = cp.tile([P, n_chunks], fp16)
            nc.sync.dma_start(winrel_sb[:], winrel_d[:])
            iota_sb = cp.tile([P, IOTA_B, QW], fp16)
            nc.sync.dma_start(iota_sb[:], iota_d[:])
            wrel_sb = cp.tile([P, L * 8 * P], bf16)
            nc.sync.dma_start(wrel_sb[:], wrelT[:])
            wroot_sb = cp.tile([P, L * 3 * P], bf16)
            nc.sync.dma_start(wroot_sb[:], wrootT[:])
            brel_sb = cp.tile([P, L * 3], f32)
            nc.sync.dma_start(brel_sb[:], brel[:])
            btw1_sb = cp.tile([P, P], bf16)
            nc.sync.dma_start(btw1_sb[:], btw1T[:])
            btw2_sb = cp.tile([P, P], bf16)
            nc.sync.dma_start(btw2_sb[:], btw2T[:])
            btb_sb = cp.tile([P, 2], f32)
            nc.sync.dma_start(btb_sb[:], btb[:])
            encb_sb = cp.tile([P, 3], f32)
            nc.sync.dma_start(encb_sb[:], encb[:])
            encw_sb = {}
            for dt_ in ("base", "joint", "foot"):
                t = cp.tile([P, encw[dt_].shape[1]], bf16, name=f"encw_{dt_}")
                nc.sync.dma_start(t[:], encw[dt_][:])
                encw_sb[dt_] = t
            ident = cp.tile([P, P], bf16)
            make_identity(nc, ident[:])

            # ---------------- DRAM internals
            agout = []
            for rnd in range(L):
                agout.append(
                    {
                        dt_: dram.tile(
                            [NTYPE[dt_], P], bf16, name=f"ago_{dt_}{rnd}",
                            addr_space="Shared",
                        )
                        for dt_ in AG_TYPES[rnd]
                    }
                )
            agin = {
                "base": dram.tile([SB_B, P], bf16, name="agi_b"),
                "joint": dram.tile([SB_J, P], bf16, name="agi_j"),
                "foot": dram.tile([SB_F, P], bf16, name="agi_f"),
            }

            # ---------------- helper: transpose 2048 rows into agin; on the
            # piece boundary, AllGather the 4096-row piece into its rank-major
            # piece table (pipelines comm through the layer)
            def ship_block(round_idx, dt_, lo, hi):
                seg = SEGOFF[dt_]
                for wt in range(lo // P, hi // P):
                    trp = psy.tile([P, P], bf16, tag="y", name=f"trp{round_idx}{dt_}{wt}")
                    nc.tensor.transpose(
                        out=trp[:],
                        in_=xT[:, seg + wt * P : seg + (wt + 1) * P],
                        identity=ident[:],
                    )
                    trs = sb.tile([P, P], bf16, tag="trs", bufs=3)
                    nc.scalar.copy(out=trs[:], in_=trp[:])
                    nc.sync.dma_start(
                        agin[dt_][wt * P : (wt + 1) * P, :], trs[:]
                    )
                if hi != SLABSZ[dt_] or os.environ.get("HGNN_NO_CC"):
                    return
                nc.gpsimd.collective_compute(
                    "AllGather",
                    mybir.AluOpType.bypass,
                    replica_groups=[list(range(NCORES))],
                    ins=[agin[dt_][:].opt()],
                    outs=[agout[round_idx][dt_][:].opt()],
                )

            # ---------------- encoder (order: joint, foot, base for AG timing)
            FP = {"base": FBp, "joint": FJp, "foot": FFp}
            for dt_ in ["joint", "base", "foot"]:
                Fp, seg = FP[dt_], SEGOFF[dt_]
                ntiles = SLABSZ[dt_] // QW
                bcol = {"base": 0, "joint": 1, "foot": 2}[dt_]
                for nt in range(ntiles):
                    acc = psn.tile([P, QW], f32, tag="news", bufs=2)
                    for fc in range(Fp // P):
                        rhs = sb.tile([P, QW], bf16, tag="encrhs", bufs=3)
                        nc.sync.dma_start(
                            rhs[:], xraw[dt_][fc * P : (fc + 1) * P, nt * QW : (nt + 1) * QW]
                        )
                        nc.tensor.matmul(
                            out=acc[:],
                            lhsT=encw_sb[dt_][:, fc * P : (fc + 1) * P],
                            rhs=rhs[:],
                            start=(fc == 0),
                            stop=(fc == Fp // P - 1),
                        )
                    nc.scalar.activation(
                        out=xT[:, seg + nt * QW : seg + (nt + 1) * QW],
                        in_=acc[:],
                        func=mybir.ActivationFunctionType.Relu,
                        bias=encb_sb[:, bcol : bcol + 1],
                    )
                if True:
                    for lo in range(0, SLABSZ[dt_], BLK):
                        ship_block(0, dt_, lo, lo + BLK)

            # ---------------- layers
            for l in range(L):
                for dt_ in DSTS_PER_LAYER[l]:
                    wlist = wave_sets[dt_]
                    seg = SEGOFF[dt_]
                    dcol = {"base": 0, "joint": 1, "foot": 2}[dt_]
                    wroot = wroot_sb[:, (l * 3 + dcol) * P : (l * 3 + dcol + 1) * P]
                    bias = brel_sb[:, l * 3 + dcol : l * 3 + dcol + 1]
                    for wave in wlist:
                        # gathers for this wave
                        msg = sb.tile(
                            [P, max_wave_nch, P], bf16, tag="msg", bufs=2,
                            name=f"m{l}{dt_}{wave['blocks'][0]}",
                        )
                        for gi, (tab, icol, n, slot0) in enumerate(wave["gathers"]):
                            if tab == 0:
                                src_ap = agout[l]["base"][:]
                            elif tab == 4:
                                src_ap = agout[l]["foot"][:]
                            else:
                                t3 = tab - 1
                                src_ap = agout[l]["joint"][
                                    t3 * THIRD : (t3 + 1) * THIRD, :
                                ]
                            if os.environ.get("HGNN_NO_GATHER"):
                                continue
                            nc.gpsimd.dma_gather(
                                out_ap=msg[:, slot0 : slot0 + n // P, :],
                                in_ap=src_ap,
                                idxs_ap=idxs_sb[:, icol : icol + n // 16],
                                num_idxs=n,
                                num_idxs_reg=n,
                                elem_size=P,
                                transpose=False,
                                single_packet=False,
                                queue_num=gather_q[0] % NQUEUES,
                            )
                            gather_q[0] += 1
                        # compute per (block, quarter)
                        for (b, q, wc0, nq, qwork) in wave["comp"]:
                            base_col = seg + b * BLK + q * QW
                            news = psn.tile(
                                [P, QW], f32, tag="news", bufs=2,
                                name=f"n{l}{dt_}{b}{q}",
                            )
                            nc.tensor.matmul(
                                out=news[:],
                                lhsT=wroot,
                                rhs=xT[:, base_col : base_col + QW],
                                start=True,
                                stop=False,
                            )
                            if qwork:
                                # batched R builds over contiguous winrel cols
                                Rq = sb.tile(
                                    [P, max_q_nch, QW], bf16, tag="R", bufs=2,
                                    name=f"R{l}{dt_}{b}{q}",
                                )
                                for i0 in range(0, nq, IOTA_B):
                                    i1 = min(i0 + IOTA_B, nq)
                                    nc.vector.tensor_tensor(
                                        out=Rq[:, i0:i1, :],
                                        in0=winrel_sb[:, wc0 + i0 : wc0 + i1]
                                        .unsqueeze(2)
                                        .to_broadcast([P, i1 - i0, QW]),
                                        in1=iota_sb[:, : i1 - i0, :],
                                        op=mybir.AluOpType.is_equal,
                                    )
                            for (slot, ti, chunks) in qwork:
                                agg = psn.tile(
                                    [P, QW], f32, tag=f"agg{slot}", bufs=1,
                                    name=f"a{l}{dt_}{b}{q}{slot}",
                                )
                                for k, (mslot, wcol) in enumerate(chunks):
                                    nc.tensor.matmul(
                                        out=agg[:],
                                        lhsT=msg[:, mslot, :],
                                        rhs=Rq[:, wcol - wc0, :],
                                        start=(k == 0),
                                        stop=False,
                                    )
                                aggsb = sb.tile(
                                    [P, QW], bf16, tag="aggsb", bufs=4
                                )
                                if ti in MEAN_T:
                                    dv = sb.tile(
                                        [P, QW], bf16, tag="dinv", bufs=2
                                    )
                                    dc0 = MEAN_T[ti] * SB_B + b * BLK + q * QW
                                    nc.sync.dma_start(
                                        dv[:], dinv_d[:, dc0 : dc0 + QW]
                                    )
                                    nc.vector.tensor_tensor(
                                        out=aggsb[:],
                                        in0=agg[:],
                                        in1=dv[:],
                                        op=mybir.AluOpType.mult,
                                    )
                                else:
                                    nc.scalar.copy(out=aggsb[:], in_=agg[:])
                                nc.tensor.matmul(
                                    out=news[:],
                                    lhsT=wrel_sb[:, (l * 8 + ti) * P : (l * 8 + ti + 1) * P],
                                    rhs=aggsb[:],
                                    start=False,
                                    stop=False,
                                )
                            # eviction
                            cols = slice(base_col, base_col + QW)
                            if dt_ != "base":
                                tmp = sb.tile([P, QW], bf16, tag="ev", bufs=3)
                                nc.scalar.activation(
                                    out=tmp[:],
                                    in_=news[:],
                                    func=mybir.ActivationFunctionType.Relu,
                                    bias=bias,
                                )
                                nc.vector.tensor_tensor(
                                    out=xT[:, cols],
                                    in0=xT[:, cols],
                                    in1=tmp[:],
                                    op=mybir.AluOpType.add,
                                )
                            else:
                                nb_ = sb.tile([P, QW], bf16, tag="ev", bufs=3)
                                nc.scalar.activation(
                                    out=nb_[:],
                                    in_=news[:],
                                    func=mybir.ActivationFunctionType.Identity,
                                    bias=bias,
                                )
                                t1p = psy.tile([P, QW], f32, tag="y", name=f"t1{l}{b}{q}")
                                nc.tensor.matmul(
                                    out=t1p[:], lhsT=btw1_sb[:], rhs=nb_[:],
                                    start=True, stop=True,
                                )
                                t1s = sb.tile([P, QW], bf16, tag="ev2", bufs=3)
                                nc.scalar.activation(
                                    out=t1s[:], in_=t1p[:],
                                    func=mybir.ActivationFunctionType.Relu,
                                    bias=btb_sb[:, 0:1],
                                )
                                t2p = psy.tile([P, QW], f32, tag="y", name=f"t2{l}{b}{q}")
                                nc.tensor.matmul(
                                    out=t2p[:], lhsT=btw2_sb[:], rhs=t1s[:],
                                    start=True, stop=True,
                                )
                                t2s = sb.tile([P, QW], bf16, tag="ev2", bufs=3)
                                nc.scalar.activation(
                                    out=t2s[:], in_=t2p[:],
                                    func=mybir.ActivationFunctionType.Identity,
                                    bias=btb_sb[:, 1:2],
                                )
                                if l < L - 1:
                                    nc.vector.tensor_tensor(
                                        out=xT[:, cols],
                                        in0=xT[:, cols],
                                        in1=t2s[:],
                                        op=mybir.AluOpType.add,
                                    )
                                else:
                                    fin = sb.tile([P, QW], bf16, tag="ev", bufs=3)
                                    nc.vector.tensor_tensor(
                                        out=fin[:],
                                        in0=xT[:, cols],
                                        in1=t2s[:],
                                        op=mybir.AluOpType.add,
                                    )
                                    for wt in range(QW // P):
                                        ftp = psy.tile([P, P], bf16, tag="y", name=f"f{b}{q}{wt}")
                                        nc.tensor.transpose(
                                            out=ftp[:],
                                            in_=fin[:, wt * P : (wt + 1) * P],
                                            identity=ident[:],
                                        )
                                        fts = sb.tile([P, P], f32, tag="fts", bufs=3)
                                        nc.scalar.copy(out=fts[:], in_=ftp[:])
                                        r0 = b * BLK + q * QW + wt * P
                                        nc.sync.dma_start(
                                            out_xb[r0 : r0 + P, :], fts[:]
                                        )
                    if l < L - 1 and dt_ in AG_TYPES[l + 1]:
                        for lo in range(0, SLABSZ[dt_], BLK):
                            ship_block(l + 1, dt_, lo, lo + BLK)

    return dram_in


# ---------------------------------------------------------------- main entry
def kernel(**inputs):
    xb = np.asarray(inputs["x_base"], np.float32)
    xj = np.asarray(inputs["x_joint"], np.float32)
    xf = np.asarray(inputs["x_foot"], np.float32)

    wave_sets, core_arrays, n_chunks, idxcols = _build_schedule(inputs)
    max_wave_nch = max(w["nch"] for ws in wave_sets.values() for w in ws)
    max_q_nch = max(
        nq for ws in wave_sets.values() for w in ws
        for (b, q, wc0, nq, qwork) in w["comp"]
    )

    nc = bacc.Bacc("TRN2", target_bir_lowering=False, debug=False, num_devices=NCORES,
                   num_swdge_queues=NQUEUES)
    _build(nc, wave_sets, n_chunks, idxcols, max_wave_nch, max_q_nch)
    nc.compile()

    # ---- per-core inputs
    def padT(x, Fp):
        out = np.zeros((Fp, x.shape[0]), np_bf16)
        out[: x.shape[1]] = np.ascontiguousarray(x.T).astype(np_bf16)
        return out

    def enc_pack(W, Fp):
        WT = np.zeros((Fp, P), np.float32)
        WT[: W.shape[1]] = W.T
        return (
            WT.reshape(Fp // P, P, P).transpose(1, 0, 2).reshape(P, Fp).astype(np_bf16)
        )

    wrel = np.asarray(inputs["conv_Wrel"], np.float32)   # [L, 8, H, H]
    wroot = np.asarray(inputs["conv_Wroot"], np.float32)
    brel = np.asarray(inputs["conv_brel"], np.float32)   # [L, 8, H]
    wrelT = (
        wrel.transpose(0, 1, 3, 2).reshape(L * 8, P, P).transpose(1, 0, 2).reshape(P, L * 8 * P)
    ).astype(np_bf16)
    wrootT = np.zeros((P, L * 3 * P), np.float32)
    brelsum = np.zeros((P, L * 3), np.float32)
    for l in range(L):
        for di, dt_ in enumerate(["base", "joint", "foot"]):
            wsum = np.zeros((P, P), np.float32)
            bsum = np.zeros(P, np.float32)
            for ti, (nm, st, d2, mn) in enumerate(ETYPES):
                if d2 == dt_:
                    wsum += wroot[l, ti]
                    bsum += brel[l, ti]
            wrootT[:, (l * 3 + di) * P : (l * 3 + di + 1) * P] = wsum.T
            brelsum[:, l * 3 + di] = bsum
    wrootT = wrootT.astype(np_bf16)

    encb = np.stack(
        [
            np.asarray(inputs["enc_b_base"], np.float32),
            np.asarray(inputs["enc_b_joint"], np.float32),
            np.asarray(inputs["enc_b_foot"], np.float32),
        ],
        axis=1,
    )
    btb = np.stack(
        [
            np.asarray(inputs["bt_b1"], np.float32),
            np.asarray(inputs["bt_b2"], np.float32),
        ],
        axis=1,
    )
    iota_tiled = np.tile(
        np.arange(QW, dtype=np.float16), IOTA_B
    ).reshape(1, IOTA_B, QW).repeat(P, 0)

    common = {
        "encWT_base": enc_pack(np.asarray(inputs["enc_W_base"], np.float32), FBp),
        "encWT_joint": enc_pack(np.asarray(inputs["enc_W_joint"], np.float32), FJp),
        "encWT_foot": enc_pack(np.asarray(inputs["enc_W_foot"], np.float32), FFp),
        "enc_b": encb,
        "wrelT": wrelT,
        "wrootT": wrootT,
        "brelsum": brelsum,
        "btW1T": np.asarray(inputs["bt_W1"], np.float32).T.astype(np_bf16).copy(),
        "btW2T": np.asarray(inputs["bt_W2"], np.float32).T.astype(np_bf16).copy(),
        "btb": btb,
        "iotat": iota_tiled,
    }

    in_maps = []
    for c in range(NCORES):
        m = dict(common)
        m["xT_base"] = padT(xb[c * SB_B : (c + 1) * SB_B], FBp)
        m["xT_joint"] = padT(xj[c * SB_J : (c + 1) * SB_J], FJp)
        m["xT_foot"] = padT(xf[c * SB_F : (c + 1) * SB_F], FFp)
        m["idxs"] = core_arrays[c]["idxs"]
        m["winrel"] = core_arrays[c]["winrel"]
        m["dinv"] = core_arrays[c]["dinv"]
        in_maps.append(m)

    trace = bool(os.environ.get("HGNN_TRACE"))
    res = run_bass_kernel_spmd(
        nc, in_maps, core_ids=list(range(NCORES)), trace=trace
    )
    if res.exec_time_ns is not None:
        print(f"HW exec time: {res.exec_time_ns} ns", flush=True)
    xbase_fin = np.concatenate(
        [res.results[c]["out_xbase"] for c in range(NCORES)], axis=0
    )  # [32768, 128] fp32

    # host decoder (tiny)
    dec_W = np.asarray(inputs["dec_W"], np.float32)
    dec_b = np.asarray(inputs["dec_b"], np.float32)
    coeff_lin = np.asarray(inputs["coeff_lin"], np.float32)
    coeff_ang = np.asarray(inputs["coeff_ang"], np.float32)
    bs = NB // 4
    out = xbase_fin.reshape(bs, 4 * H) @ dec_W.T + dec_b
    xr = out.reshape(bs, 4, 6)
    x_lin = (xr[:, :, :3].reshape(bs, 12) * coeff_lin).reshape(bs, 4, 3)
    x_ang = (xr[:, :, 3:].reshape(bs, 12) * coeff_ang).reshape(bs, 4, 3)
    return np.concatenate([x_lin, x_ang], axis=-1).reshape(bs, 24).astype(np.float32)
